# revision 1
# baseline (speedup 1.0000x reference)
"""Cross-attention layer on 8 Trainium2 NeuronCores (Bass/Tile SPMD).

Sharding: tensor-parallel over heads. Each core owns 4 of the 32 heads:
it projects Q^T/K^T/V for its heads (bf16 matmuls, fp32 accumulate),
runs masked softmax attention in transposed layout (scores^T so the
softmax v-reduction is a PE ones-matmul and no attn transpose is ever
needed), then AllToAlls redistribute ctx^T from head-sharded to
token-sharded so every core runs the output projection + residual +
LayerNorm for its own 256-token slice. Host concatenates the 8 slices.

Overlap structure (v2): phase B iterates head-PAIR outer so the A2A is
split into two half-collectives; chunk 0 flies while heads 2-3 compute,
chunk 1 overlaps the first o-proj contraction pass. Phase C accumulates
the o-proj in SBUF fp32 across the two passes (16 contraction tiles
each), folding the residual in pass 0 and the LayerNorm statistics
(sum via stt accum, sum-of-squares via Act Square accum; var =
E[x^2]-mu^2) into pass 1, so the post-matmul tail is just
rsqrt + 512-col normalize+store chunks. Wo streams during phase B
(its DMAs have no deps and the DMA engines are idle then).

Numerics: matmul inputs bf16 (error ~1e-3 of output scale, validated
against the fp32 reference), all accumulation fp32, softmax without
max-subtraction (scores ~N(0,1), exp can't overflow), mask folded into
the exp bias, 1/sqrt(hd) folded into Wq on host, bv folded into an
effective bo on host (rows of attn sum to 1), residual+LN in fp32.
"""
import sys

sys.path.insert(0, "/opt/trn_rl_repo")

import numpy as np
import ml_dtypes

import concourse.bacc as bacc
import concourse.mybir as mybir
import concourse.tile as tile
from concourse.bass_utils import run_bass_kernel_spmd

BF16 = ml_dtypes.bfloat16

NCORES = 8
P = 128            # partitions / head dim / k-tile
H = 4096
KT = H // P        # 32 k-tiles along any H contraction
NH = 32
NHL = NH // NCORES  # 4 local heads
CW = NHL * P       # 512 local c-columns
B = 2
LB = 1024          # tokens per batch
L2 = B * LB        # 2048 total tokens
TL = L2 // NCORES  # 256 tokens per core after A2A
QW = 512           # token-quarter width in phase A
NQ = L2 // QW      # 4
NVT = L2 // P      # 16 v tiles total (8 per batch)
MSK = -1e30
MCW = 512          # o-proj output-column chunk
NMC = H // MCW     # 8
NPAIR = 2          # head pairs per core (A2A chunks)
GPP = NCORES * 2   # contraction g-tiles per o-proj pass

_CACHE = {}

F32 = mybir.dt.float32
BF = mybir.dt.bfloat16


def _build(debug=False):
    nc = bacc.Bacc("TRN2", target_bir_lowering=False, debug=False,
                   num_devices=NCORES)

    hidT_d = nc.dram_tensor("hidT", [H, L2], BF, kind="ExternalInput")
    visT_d = nc.dram_tensor("visT", [H, L2], BF, kind="ExternalInput")
    wqT_d = nc.dram_tensor("wqT", [H, CW], BF, kind="ExternalInput")
    wkT_d = nc.dram_tensor("wkT", [H, CW], BF, kind="ExternalInput")
    wvT_d = nc.dram_tensor("wvT", [H, CW], BF, kind="ExternalInput")
    woT_d = nc.dram_tensor("woT", [H, H], BF, kind="ExternalInput")
    bqT_d = nc.dram_tensor("bqT", [P, NHL], F32, kind="ExternalInput")
    bkT_d = nc.dram_tensor("bkT", [P, NHL], F32, kind="ExternalInput")
    mskb_d = nc.dram_tensor("mskb", [P, B * 8], F32, kind="ExternalInput")
    hb_d = nc.dram_tensor("hb", [TL, H], BF, kind="ExternalInput")
    g_d = nc.dram_tensor("g", [P, H], BF, kind="ExternalInput")
    bta_d = nc.dram_tensor("bta", [P, H], BF, kind="ExternalInput")
    out_d = nc.dram_tensor("out", [TL, H], F32, kind="ExternalOutput")

    with tile.TileContext(nc) as tc:
        with tc.tile_pool(name="persist", bufs=1) as pers, \
             tc.tile_pool(name="dram", bufs=1, space="DRAM") as dram:

            pqkv = tc.alloc_tile_pool(name="pqkv", bufs=1)
            qT_sb = pqkv.tile([P, NHL * L2], BF)     # Q^T/sqrt(hd): [hd, (h, l)]
            kT_sb = pqkv.tile([P, NHL * L2], BF)     # K^T: [hd, (h, v)]
            v_sb = pqkv.tile([P, NVT * CW], BF)      # V: [v, (vt, c)]
            # ctx^T normalized, split per head pair: [hd, (hh, l)]
            ctxT_sb = [pqkv.tile([P, 2 * L2], BF, name=f"ctxT{i}")
                       for i in range(NPAIR)]
            bqT_sb = pers.tile([P, NHL], F32)
            bkT_sb = pers.tile([P, NHL], F32)
            mskb_sb = pers.tile([P, B * 8], F32)
            ones_bf = pers.tile([P, 1], BF)
            ones_f32 = pers.tile([1, P], F32)
            nc.sync.dma_start(out=bqT_sb[:], in_=bqT_d[:])
            nc.sync.dma_start(out=bkT_sb[:], in_=bkT_d[:])
            nc.sync.dma_start(out=mskb_sb[:], in_=mskb_d[:])
            nc.vector.memset(ones_bf[:], 1.0)
            nc.vector.memset(ones_f32[:], 1.0)

            # ---------------- Phase A: Q^T, K^T, V projections ----------------
            with tc.tile_pool(name="phaseA", bufs=2) as pa, \
                 tc.tile_pool(name="psA", bufs=6, space="PSUM") as psA:

                def load_w(dram_t, tag="wproj"):
                    w_sb = pa.tile([P, KT * CW], BF, tag=tag, name="w_sb")
                    nc.sync.dma_start(
                        out=w_sb[:].rearrange("p (kt c) -> p kt c", kt=KT),
                        in_=dram_t[:].rearrange("(kt p) c -> p kt c", p=P))
                    return w_sb

                wq_sb = load_w(wqT_d)

                def proj_qk(x_dram, w_sb, b_sb, dst_sb):
                    for q in range(NQ):
                        xT = pa.tile([P, KT * QW], BF, tag="xT")
                        nc.sync.dma_start(
                            out=xT[:].rearrange("p (kt l) -> p kt l", kt=KT),
                            in_=x_dram[:, q * QW:(q + 1) * QW]
                                .rearrange("(kt p) l -> p kt l", p=P))
                        for h in range(NHL):
                            ps = psA.tile([P, QW], F32, tag="psA")
                            for kt in range(KT):
                                nc.tensor.matmul(
                                    ps[:],
                                    w_sb[:, kt * CW + h * P: kt * CW + (h + 1) * P],
                                    xT[:, kt * QW:(kt + 1) * QW],
                                    start=(kt == 0), stop=(kt == KT - 1))
                            nc.vector.tensor_scalar_add(
                                dst_sb[:, h * L2 + q * QW: h * L2 + (q + 1) * QW],
                                ps[:], b_sb[:, h:h + 1])

                proj_qk(hidT_d, wq_sb, bqT_sb, qT_sb)
                # K/V weights load behind the Q-phase xT tiles in the SP
                # queue so the first matmul isn't gated on 12.6MB of weights
                wk_sb = load_w(wkT_d)
                wv_sb = load_w(wvT_d)
                proj_qk(visT_d, wk_sb, bkT_sb, kT_sb)

                # V in natural [v, c] layout: lhsT = visT tile, rhs = WvT
                for q in range(NQ):
                    xT = pa.tile([P, KT * QW], BF, tag="xT")
                    nc.sync.dma_start(
                        out=xT[:].rearrange("p (kt l) -> p kt l", kt=KT),
                        in_=visT_d[:, q * QW:(q + 1) * QW]
                            .rearrange("(kt p) l -> p kt l", p=P))
                    for vt in range(4):
                        g_vt = q * 4 + vt
                        ps = psA.tile([P, CW], F32, tag="psA")
                        for kt in range(KT):
                            nc.tensor.matmul(
                                ps[:],
                                xT[:, kt * QW + vt * P: kt * QW + (vt + 1) * P],
                                wv_sb[:, kt * CW:(kt + 1) * CW],
                                start=(kt == 0), stop=(kt == KT - 1))
                        nc.scalar.copy(
                            out=v_sb[:, g_vt * CW:(g_vt + 1) * CW], in_=ps[:])

            # Pools that must be resident before phase B so their DMAs can
            # stream during it: o-proj weight double-buffer, x accumulator,
            # residual rows.
            pcw = tc.alloc_tile_pool(name="pcw", bufs=3)
            pacc = tc.alloc_tile_pool(name="pacc", bufs=1)
            x_sb = [pacc.tile([P, H], F32, name=f"x_sb{lt}") for lt in range(2)]
            hb_sb = [pacc.tile([P, H], BF, name=f"hb_sb{lt}") for lt in range(2)]
            for lt in range(2):
                nc.sync.dma_start(out=hb_sb[lt][:],
                                  in_=hb_d[lt * P:(lt + 1) * P, :])

            def load_wo(pair, mc, eng=None):
                # o-proj weight chunk: g-tile order (j, hh);
                # c-row = j*512 + (pair*2+hh)*128 + p. Even chunks load on
                # the SP queue, odd on Act, so a WAR-gated load only convoys
                # every other chunk.
                if eng is None:
                    eng = nc.sync
                wo_sb = pcw.tile([P, GPP * MCW], BF, tag="wo", name="wo_sb")
                for j in range(NCORES):
                    eng.dma_start(
                        out=wo_sb[:, j * 2 * MCW:(j + 1) * 2 * MCW]
                            .rearrange("p (hh m) -> p hh m", hh=2),
                        in_=woT_d[j * CW + pair * 2 * P:
                                  j * CW + (pair * 2 + 2) * P,
                                  mc * MCW:(mc + 1) * MCW]
                            .rearrange("(hh p) m -> p hh m", p=P))
                return wo_sb

            # prefetch ahead of the staging DMAs in the SP queue so these
            # stream during phase B
            wo_pre = [load_wo(0, 0, nc.sync), load_wo(0, 1, nc.sync)]

            # ---------------- Phase B: attention, head-pair outer ----------------
            a2a_in = [dram.tile([NCORES, 2 * P, TL], BF, name=f"a2ai{i}")
                      for i in range(NPAIR)]
            a2a_out = [dram.tile([NCORES, 2 * P, TL], BF, name=f"a2ao{i}")
                       for i in range(NPAIR)]
            with tc.tile_pool(name="phaseB", bufs=2) as pb, \
                 tc.tile_pool(name="psB", bufs=2, space="PSUM") as psB:
                for pair in range(NPAIR):
                    for hh in range(2):
                        h = pair * 2 + hh
                        for b in range(B):
                            for lh in range(2):
                                qcol = h * L2 + b * LB + lh * QW
                                ccol = hh * L2 + b * LB + lh * QW
                                attnT = pb.tile([P, 8 * QW], BF, tag="attnT",
                                                bufs=3)
                                rs_ps = psB.tile([1, QW], F32, tag="rs")
                                for vb in range(8):
                                    sc_ps = psB.tile([P, QW], F32, tag="sc")
                                    nc.tensor.matmul(
                                        sc_ps[:],
                                        kT_sb[:, h * L2 + b * LB + vb * P:
                                              h * L2 + b * LB + (vb + 1) * P],
                                        qT_sb[:, qcol: qcol + QW],
                                        start=True, stop=True)
                                    mcol = b * 8 + vb
                                    nc.scalar.activation(
                                        attnT[:, vb * QW:(vb + 1) * QW], sc_ps[:],
                                        mybir.ActivationFunctionType.Exp,
                                        bias=mskb_sb[:, mcol:mcol + 1], scale=1.0)
                                    nc.tensor.matmul(
                                        rs_ps[:], ones_bf[:],
                                        attnT[:, vb * QW:(vb + 1) * QW],
                                        start=(vb == 0), stop=(vb == 7))
                                rcp_sb = pb.tile([1, QW], F32, tag="rcp")
                                nc.vector.reciprocal(rcp_sb[:], rs_ps[:])
                                rcp_ps = psB.tile([P, QW], F32, tag="rcpp")
                                nc.tensor.matmul(rcp_ps[:], ones_f32[:], rcp_sb[:],
                                                 start=True, stop=True)
                                rcp_rep = pb.tile([P, QW], F32, tag="rcprep")
                                nc.scalar.copy(out=rcp_rep[:], in_=rcp_ps[:])
                                ctx_ps = psB.tile([P, QW], F32, tag="ctx")
                                for vb in range(8):
                                    nc.tensor.matmul(
                                        ctx_ps[:],
                                        v_sb[:, (b * 8 + vb) * CW + h * P:
                                             (b * 8 + vb) * CW + (h + 1) * P],
                                        attnT[:, vb * QW:(vb + 1) * QW],
                                        start=(vb == 0), stop=(vb == 7))
                                nc.vector.tensor_tensor(
                                    out=ctxT_sb[pair][:, ccol: ccol + QW],
                                    in0=ctx_ps[:], in1=rcp_rep[:],
                                    op=mybir.AluOpType.mult)
                    # stage + launch this pair's half-A2A while the next pair
                    # (or phase C pass 0) computes
                    for hh in range(2):
                        nc.sync.dma_start(
                            out=a2a_in[pair][:, hh * P:(hh + 1) * P, :]
                                .rearrange("j p l -> p j l"),
                            in_=ctxT_sb[pair][:, hh * L2:(hh + 1) * L2]
                                .rearrange("p (j l) -> p j l", j=NCORES))
                    nc.gpsimd.collective_compute(
                        "AllToAll", mybir.AluOpType.bypass,
                        replica_groups=[list(range(NCORES))],
                        ins=[a2a_in[pair][:]], outs=[a2a_out[pair][:]])

            # ---------------- Phase C: o-proj passes, residual + LN ----------------
            with tc.tile_pool(name="phaseC", bufs=2) as pc, \
                 tc.tile_pool(name="psC", bufs=6, space="PSUM") as psC, \
                 tc.tile_pool(name="phaseC1", bufs=1) as pc1:
                musum_col = [pc1.tile([P, NMC], F32, name=f"mus{lt}")
                             for lt in range(2)]
                ssq_col = [pc1.tile([P, NMC], F32, name=f"ssq{lt}")
                           for lt in range(2)]
                g_sb = pc1.tile([P, H], BF)
                bta_sb = pc1.tile([P, H], BF)

                for pair in range(NPAIR):
                    # received ctx^T for my 256 tokens, this pair's heads:
                    # g-tile order (j, hh); c-row = j*512 + (pair*2+hh)*128 + p
                    octxT = pc1.tile([P, GPP * TL], BF, name=f"octx{pair}")
                    for j in range(NCORES):
                        nc.scalar.dma_start(
                            out=octxT[:, j * 2 * TL:(j + 1) * 2 * TL]
                                .rearrange("p (hh l) -> p hh l", hh=2),
                            in_=a2a_out[pair][j]
                                .rearrange("(hh p) l -> p hh l", p=P))
                    if pair == 1:
                        # needed only by the LN epilogue; keep them behind the
                        # latency-critical octxT loads on the Act queue
                        nc.scalar.dma_start(out=g_sb[:], in_=g_d[:])
                        nc.scalar.dma_start(out=bta_sb[:], in_=bta_d[:])
                    for mc in range(NMC):
                        if pair == 0 and mc < 2:
                            wo_sb = wo_pre[mc]
                        else:
                            wo_sb = load_wo(pair, mc)
                        for lt in range(2):
                            po = psC.tile([P, MCW], F32, tag="po")
                            for g in range(GPP):
                                nc.tensor.matmul(
                                    po[:],
                                    octxT[:, g * TL + lt * P: g * TL + lt * P + P],
                                    wo_sb[:, g * MCW:(g + 1) * MCW],
                                    start=(g == 0), stop=(g == GPP - 1))
                            xc = x_sb[lt][:, mc * MCW:(mc + 1) * MCW]
                            if pair == 0:
                                nc.vector.tensor_tensor(
                                    out=xc, in0=po[:],
                                    in1=hb_sb[lt][:, mc * MCW:(mc + 1) * MCW],
                                    op=mybir.AluOpType.add)
                            else:
                                nc.vector.scalar_tensor_tensor(
                                    out=xc, in0=po[:], scalar=1.0, in1=xc,
                                    op0=mybir.AluOpType.mult,
                                    op1=mybir.AluOpType.add,
                                    accum_out=musum_col[lt][:, mc:mc + 1])
                                sq = pc.tile([P, MCW], F32, tag="sq")
                                nc.scalar.activation(
                                    sq[:], xc,
                                    mybir.ActivationFunctionType.Square,
                                    accum_out=ssq_col[lt][:, mc:mc + 1])

                # LayerNorm epilogue: var = E[x^2] - mu^2, then per-chunk
                # out = x*rstd*g + (beta - mu*rstd*g)
                epst = pers.tile([P, 1], F32)
                nc.vector.memset(epst[:], 1e-5)
                for lt in range(2):
                    musum = pc.tile([P, 1], F32)
                    nc.vector.reduce_sum(musum[:], musum_col[lt][:],
                                         axis=mybir.AxisListType.X)
                    ssq = pc.tile([P, 1], F32)
                    nc.vector.reduce_sum(ssq[:], ssq_col[lt][:],
                                         axis=mybir.AxisListType.X)
                    mu = pc.tile([P, 1], F32)
                    nc.scalar.mul(mu[:], musum[:], 1.0 / H)
                    mu2 = pc.tile([P, 1], F32)
                    nc.vector.tensor_tensor(out=mu2[:], in0=mu[:], in1=mu[:],
                                            op=mybir.AluOpType.mult)
                    bias_t = pc.tile([P, 1], F32)
                    nc.vector.tensor_tensor(out=bias_t[:], in0=epst[:],
                                            in1=mu2[:],
                                            op=mybir.AluOpType.subtract)
                    std = pc.tile([P, 1], F32)
                    nc.scalar.activation(
                        std[:], ssq[:], mybir.ActivationFunctionType.Sqrt,
                        bias=bias_t[:], scale=1.0 / H)
                    rstd = pc.tile([P, 1], F32)
                    nc.vector.reciprocal(rstd[:], std[:])
                    ms = pc.tile([P, 1], F32)
                    nc.vector.tensor_tensor(out=ms[:], in0=mu[:], in1=rstd[:],
                                            op=mybir.AluOpType.mult)
                    nc.vector.tensor_scalar(
                        out=ms[:], in0=ms[:], scalar1=-1.0, scalar2=None,
                        op0=mybir.AluOpType.mult)
                    # out = (x*rstd - mu*rstd)*g + beta, per 512-col chunk,
                    # in-place: the per-partition affine on Act (Identity),
                    # then g-mult and beta-add split ~5:3 DVE:Pool.
                    dma_eng = nc.sync if lt == 0 else nc.scalar
                    for mc in range(NMC):
                        eng = nc.vector if mc < 5 else nc.gpsimd
                        cs = slice(mc * MCW, (mc + 1) * MCW)
                        nc.scalar.activation(
                            x_sb[lt][:, cs], x_sb[lt][:, cs],
                            mybir.ActivationFunctionType.Identity,
                            bias=ms[:], scale=rstd[:])
                        eng.tensor_tensor(
                            out=x_sb[lt][:, cs], in0=x_sb[lt][:, cs],
                            in1=g_sb[:, cs], op=mybir.AluOpType.mult)
                        eng.tensor_tensor(
                            out=x_sb[lt][:, cs], in0=x_sb[lt][:, cs],
                            in1=bta_sb[:, cs], op=mybir.AluOpType.add)
                        dma_eng.dma_start(
                            out=out_d[lt * P:(lt + 1) * P, cs],
                            in_=x_sb[lt][:, cs])
            pacc.release()
            pcw.release()
            pqkv.release()

    nc.compile()
    return nc


def _prep_inputs(hidden_states, vision_features, attention_mask,
                 Wq, bq, Wk, bk, Wv, bv, Wo, bo, ln_g, ln_b):
    f = np.asarray
    hs = f(hidden_states, dtype=np.float32).reshape(L2, H)
    vf = f(vision_features, dtype=np.float32).reshape(L2, H)
    am = f(attention_mask)
    Wq, bq = f(Wq, dtype=np.float32), f(bq, dtype=np.float32)
    Wk, bk = f(Wk, dtype=np.float32), f(bk, dtype=np.float32)
    Wv, bv = f(Wv, dtype=np.float32), f(bv, dtype=np.float32)
    Wo, bo = f(Wo, dtype=np.float32), f(bo, dtype=np.float32)
    ln_g, ln_b = f(ln_g, dtype=np.float32), f(ln_b, dtype=np.float32)

    s = 1.0 / np.sqrt(P)
    hidT = np.ascontiguousarray(hs.T).astype(BF16)
    visT = np.ascontiguousarray(vf.T).astype(BF16)
    woT = np.ascontiguousarray(Wo.T).astype(BF16)
    mb = np.where(am != 0, 0.0, MSK).astype(np.float32)          # (B, LB)
    mskb = np.ascontiguousarray(
        mb.reshape(B, 8, P).transpose(2, 0, 1).reshape(P, B * 8))
    bo_eff = bo + Wo @ bv
    g_rep = np.ascontiguousarray(np.broadcast_to(ln_g, (P, H))).astype(BF16)
    b_rep = np.ascontiguousarray(np.broadcast_to(ln_b, (P, H))).astype(BF16)

    in_maps = []
    for c in range(NCORES):
        sl = slice(c * CW, (c + 1) * CW)
        in_maps.append({
            "hidT": hidT,
            "visT": visT,
            "wqT": np.ascontiguousarray((Wq[sl] * s).T).astype(BF16),
            "wkT": np.ascontiguousarray(Wk[sl].T).astype(BF16),
            "wvT": np.ascontiguousarray(Wv[sl].T).astype(BF16),
            "woT": woT,
            "bqT": np.ascontiguousarray((bq[sl] * s).reshape(NHL, P).T),
            "bkT": np.ascontiguousarray(bk[sl].reshape(NHL, P).T),
            "mskb": mskb,
            "hb": np.ascontiguousarray(hs[c * TL:(c + 1) * TL] + bo_eff).astype(BF16),
            "g": g_rep,
            "bta": b_rep,
        })
    return in_maps


def kernel(**inputs) -> np.ndarray:
    inputs.pop("_debug", False)
    if "main" not in _CACHE:
        _CACHE["main"] = _build()
    nc = _CACHE["main"]
    in_maps = _prep_inputs(**inputs)
    res = run_bass_kernel_spmd(nc, in_maps, list(range(NCORES)))
    out = np.concatenate([res.results[c]["out"] for c in range(NCORES)], axis=0)
    return out.reshape(B, LB, H)

